# revision 3
# baseline (speedup 1.0000x reference)
"""Trainium2 Bass kernel for nn_Attention_38233798869191 (v3, batch x head
sharded, all-bf16).

Sharding: core c owns batch b = c//4 and heads 4*(c%4) .. 4*(c%4)+3 (256
features). Vs the pure head-TP v2 this halves the dominant HBM traffic --
each core reads only its batch's activations (4.2 MB vs 8.4) and writes a
y-partial for only its batch (4.2 MB vs 8.4); the host all-reduce sums 4
partials per batch instead of 8. The 8 NeuronCores share HBM bandwidth, so
per-rep chip traffic is what the kernel is bound by.

Same compute pipeline as v2:
  * bf16 datapath, f32 PSUM accumulation.
  * v produced directly in natural [seq, feat] layout (lhsT = x tile).
  * p@v in natural-o form with a ones column producing the softmax
    denominator; normalize on DVE.
  * ACT does only the 128 exp instructions per rep.
  * credit-paced interleave of projection/output-projection units into the
    ACT-bound attention stream.
  * software-pipelined reps. Because every head reads every projection
    column, qT/kT/v2/M are parity double-buffered across reps so the next
    rep's projections never WAR-stall against this rep's attention.
"""

import os
import sys

import numpy as np

for _p in ("/opt/trn_rl_repo", "/root/.axon_site/_ro/trn_rl_repo"):
    if os.path.isdir(_p) and _p not in sys.path:
        sys.path.insert(0, _p)

B, S, D, H, DH = 2, 2048, 1024, 16, 64
SCALE = 1.0 / float(np.sqrt(DH))
N_CORES = 8
P = 128
NF = 256  # features per core (4 heads)
NHL = 4  # local heads

DTYPE_MODE = os.environ.get("KERNEL_DTYPE_MODE", "bf16")


def _build_nc(mode="bf16", reps=1):
    import concourse.bass as bass  # noqa: F401
    import concourse.mybir as mybir
    import concourse.tile as tile
    from concourse import bacc

    f32 = mybir.dt.float32
    elt = mybir.dt.bfloat16
    AF = mybir.ActivationFunctionType

    nc = bacc.Bacc(
        "TRN2",
        target_bir_lowering=False,
        debug=False,
        num_devices=N_CORES,
    )

    xT = nc.dram_tensor("xT", [D, S], elt, kind="ExternalInput")
    wqT = nc.dram_tensor("wqT", [D, NF], elt, kind="ExternalInput")
    wkT = nc.dram_tensor("wkT", [D, NF], elt, kind="ExternalInput")
    wvT = nc.dram_tensor("wvT", [D, NF], elt, kind="ExternalInput")
    woT = nc.dram_tensor("woT", [NF, D], elt, kind="ExternalInput")
    bqc = nc.dram_tensor("bqc", [P, 2], f32, kind="ExternalInput")  # bq*SCALE
    bkc = nc.dram_tensor("bkc", [P, 2], f32, kind="ExternalInput")
    bvr = nc.dram_tensor("bvr", [1, NF], elt, kind="ExternalInput")
    ypT = nc.dram_tensor("ypT", [D, S], elt, kind="ExternalOutput")
    osc = nc.dram_tensor("osc", [NHL, S, DH], elt)  # o natural per local head

    # DRAM views
    # d global = kc*128 + p
    xTv = xT.ap().rearrange("(kc p) s -> p kc s", p=P)  # [128, 8, 2048]
    oscM = osc.ap().rearrange("h (r k) d -> h r (k d)", r=64)  # [4, 64, 2048]

    def wview(w):
        return w.ap().rearrange("(kc p) m -> p kc m", p=P)  # [128, 8, 256]

    def woview(w):
        return w.ap().rearrange("(ft p) m -> ft p m", p=P)  # [2, 128, 1024]

    with tile.TileContext(nc) as tc:
        with tc.tile_pool(name="persist", bufs=1) as pp:
            w_sb = {}
            for name, w in (("q", wqT), ("k", wkT), ("v", wvT)):
                w_sb[name] = pp.tile([P, 8, NF], elt, tag=f"w{name}", name=f"w{name}")
                nc.sync.dma_start(w_sb[name][:], wview(w))
            woT_sb = pp.tile([P, 2, D], elt, tag="wo", name="wo")
            nc.sync.dma_start(
                woT_sb[:], woview(woT).rearrange("ft p m -> p ft m")
            )
            bias_sb = {}
            for name, bt, shape, dt_ in (
                ("q", bqc, [P, 2], f32),
                ("k", bkc, [P, 2], f32),
                ("v", bvr, [1, NF], elt),
            ):
                bias_sb[name] = pp.tile(shape, dt_, tag=f"b{name}", name=f"b{name}")
                nc.sync.dma_start(bias_sb[name][:], bt.ap())
            ones_sb = pp.tile([1, 512], elt, tag="ones", name="ones")
            nc.vector.memset(ones_sb[:], 1.0)

            # parity double-buffered activation tensors (next rep's
            # projections write parity^1 while this rep's attention reads
            # parity)
            qT_sb = [
                pp.tile([P, 2, S], elt, tag=f"qT{par}", name=f"qT{par}")
                for par in range(2)
            ]
            kT_sb = [
                pp.tile([P, 2, S], elt, tag=f"kT{par}", name=f"kT{par}")
                for par in range(2)
            ]
            # v natural: [sk 128, kc 16, head 4, 65]; col 64 == 1.0 forever
            v2 = [
                pp.tile([P, 16, NHL, 65], elt, tag=f"v{par}", name=f"v{par}")
                for par in range(2)
            ]
            for par in range(2):
                nc.vector.memset(v2[par][:], 1.0)
            # M rows [256, 2048] as 2 feature-tiles of 128 partitions
            M_sb = [
                [
                    pp.tile([P, S], elt, tag=f"M{par}{ft}", name=f"M{par}{ft}")
                    for ft in range(2)
                ]
                for par in range(2)
            ]

            with (
                tc.tile_pool(
                    name="xin", bufs=int(os.environ.get("X_BUFS", "6"))
                ) as xpool,
                # PSUM budget (8 banks), one accumulation group per bank:
                #   ps:  2 x [128,1024] f32 = 4 banks (scores)
                #   acc: 2 x [128,512] f32  = 2 banks (p@v accumulators)
                #   aux: 2 x [128,512] f32  = 2 banks (q/k/v-nat/outproj)
                tc.tile_pool(name="psum", bufs=1, space="PSUM") as psp,
                tc.tile_pool(
                    name="ptp", bufs=int(os.environ.get("PT_BUFS", "24"))
                ) as ptp,
                tc.tile_pool(name="obp", bufs=2) as obp,
                tc.tile_pool(name="rcp", bufs=4) as rcp,
                tc.tile_pool(name="ysb", bufs=3) as ysbp,
            ):
                _live = {}

                def load_x(sq):
                    x_sb = xpool.tile([P, 8, 512], elt, tag="x", name="x")
                    nc.sync.dma_start(
                        x_sb[:], xTv[:, :, sq * 512 : (sq + 1) * 512]
                    )
                    _live[("x", sq)] = x_sb

                def qk_mm(par, sq, n, ft):
                    """q or k projection, one 512-col slab, one feature tile:
                    8-step accumulation into a 1-bank PSUM tile; bias applied
                    by the DVE eviction."""
                    pacc = psp.tile([P, 512], f32, tag="aux", bufs=2, name="aux")
                    x_sb = _live[("x", sq)]
                    for kc in range(8):
                        nc.tensor.matmul(
                            pacc[:],
                            w_sb[n][:, kc, ft * P : (ft + 1) * P],
                            x_sb[:, kc, :],
                            start=(kc == 0),
                            stop=(kc == 7),
                        )
                    dst = qT_sb[par] if n == "q" else kT_sb[par]
                    nc.vector.tensor_scalar_add(
                        dst[:, ft, sq * 512 : (sq + 1) * 512],
                        pacc[:],
                        bias_sb[n][:, ft : ft + 1],
                    )

                def v_mm(par, sq, mh):
                    """v natural for sub-chunks {2*mh, 2*mh+1} of this slab:
                    two sequential [128,256] groups in one aux bank."""
                    sv = psp.tile([P, 512], f32, tag="aux", bufs=2, name="aux")
                    x_sb = _live[("x", sq)]
                    for i, m in enumerate((2 * mh, 2 * mh + 1)):
                        svm = sv[:, i * NF : (i + 1) * NF]
                        for kc in range(8):
                            nc.tensor.matmul(
                                svm,
                                x_sb[:, kc, m * P : (m + 1) * P],
                                w_sb["v"][:, kc, :],
                                start=(kc == 0),
                                stop=False,
                            )
                        nc.tensor.matmul(
                            svm,
                            ones_sb[:, 0:P],
                            bias_sb["v"][:],
                            start=False,
                            stop=True,
                        )
                    if mh == 1:
                        _live.pop(("x", sq))
                    kc0 = sq * 4 + mh * 2
                    nc.vector.tensor_copy(
                        v2[par][:, kc0 : kc0 + 2, :, 0:64],
                        sv[:].rearrange("p (m h x) -> p m h x", m=2, h=NHL),
                    )

                def proj_units(par):
                    """One batch's projections as (weight, fn) units; x loads
                    prefetched two slabs ahead."""
                    slabs = list(range(4))
                    units = [(0.1, lambda sq=sq: load_x(sq)) for sq in slabs[:2]]
                    for i, sq in enumerate(slabs):
                        for ft in range(2):
                            units.append(
                                (1.7, lambda sq=sq, ft=ft: qk_mm(par, sq, "q", ft))
                            )
                            units.append(
                                (1.7, lambda sq=sq, ft=ft: qk_mm(par, sq, "k", ft))
                            )
                        units.append((1.9, lambda sq=sq: v_mm(par, sq, 0)))
                        units.append((1.9, lambda sq=sq: v_mm(par, sq, 1)))
                        if i + 2 < 4:
                            units.append((0.1, lambda sq=slabs[i + 2]: load_x(sq)))
                    return units

                def outproj_unit(par, mo, j, evict_engine):
                    # ysb holds an mo-PAIR [128, 2, 2048] so the y-partial
                    # leaves in one DMA per pair (g = mo % 2)
                    mp, g = mo // 2, mo % 2
                    if g == 0 and j == 0:
                        _live[("y", mp)] = ysbp.tile(
                            [P, 2, 2048], elt, tag="y", name="y"
                        )
                    ysb = _live[("y", mp)]
                    py = psp.tile([P, 512], f32, tag="aux", bufs=2, name="aux")
                    for ft in range(2):
                        nc.tensor.matmul(
                            py[:],
                            woT_sb[:, ft, mo * P : (mo + 1) * P],
                            M_sb[par][ft][:, j * 512 : (j + 1) * 512],
                            start=(ft == 0),
                            stop=(ft == 1),
                        )
                    dst = ysb[:, g, j * 512 : (j + 1) * 512]
                    if evict_engine == "act":
                        nc.scalar.copy(dst, py[:])
                    else:
                        nc.vector.tensor_copy(dst, py[:])
                    if g == 1 and j == 3:
                        _live.pop(("y", mp))
                        nc.sync.dma_start(
                            ypT.ap()[
                                mp * 256 : (mp + 1) * 256, :
                            ].rearrange("(g p) s -> p g s", p=P),
                            ysb[:],
                        )

                def outproj_units(par, evict_engine="dve"):
                    return [
                        (
                            0.9,
                            lambda mo=mo, j=j: outproj_unit(par, mo, j, evict_engine),
                        )
                        for mo in range(8)
                        for j in range(4)
                    ]

                pace_s1 = float(os.environ.get("PACE_S1", "0.35"))
                pace_s2 = float(os.environ.get("PACE_S2", "0.85"))
                pace_cap = float(os.environ.get("PACE_CAP", "1.8"))
                w_scale = float(os.environ.get("PACE_W", "1.0"))

                class Pacer:
                    def __init__(self):
                        self.q = []
                        self.credit = 0.0

                    def add(self, units):
                        self.q.extend(units)

                    def slot(self, budget):
                        self.credit = min(self.credit + budget, pace_cap)
                        while self.q and self.q[0][0] * w_scale <= self.credit:
                            w, fn = self.q.pop(0)
                            self.credit -= w * w_scale
                            fn()

                    def flush(self):
                        for _, fn in self.q:
                            fn()
                        self.q.clear()
                        self.credit = 0.0

                def attention_head(par, hl, pacer):
                    ft = hl // 2
                    hsl = slice((hl % 2) * 64, (hl % 2) * 64 + 64)
                    # one o tile per head (both query halves) -> single osc
                    # DMA + single M DMA per head
                    ob = obp.tile([P, 16, DH], elt, tag="ob", name="ob")
                    for sqh in range(2):  # halves of 1024 queries
                        sq0 = sqh * 1024
                        ptiles = []
                        for kc in range(16):
                            k0 = kc * P
                            ps = psp.tile(
                                [P, 1024], f32, tag="ps", bufs=2, name="ps"
                            )
                            for half in range(2):
                                nc.tensor.matmul(
                                    ps[:, half * 512 : (half + 1) * 512],
                                    kT_sb[par][hsl, ft, k0 : k0 + P],
                                    qT_sb[par][
                                        hsl,
                                        ft,
                                        sq0 + half * 512 : sq0 + (half + 1) * 512,
                                    ],
                                    start=True,
                                    stop=True,
                                )
                            ptile = ptp.tile([P, 1024], elt, tag="pt", name="pt")
                            nc.scalar.activation(ptile[:], ps[:], AF.Exp)
                            ptiles.append(ptile)
                            pacer.slot(pace_s1)
                        for ch in range(8):
                            acc = psp.tile(
                                [P, 512], f32, tag="acc", bufs=2, name="acc"
                            )
                            for kc in range(16):
                                nc.tensor.matmul(
                                    acc[:, 0:65],
                                    ptiles[kc][:, ch * P : (ch + 1) * P],
                                    v2[par][:, kc, hl, :],
                                    start=(kc == 0),
                                    stop=(kc == 15),
                                )
                            rc = rcp.tile([P, 1], f32, tag="rc", name="rc")
                            nc.vector.reciprocal(rc[:], acc[:, 64:65])
                            nc.vector.tensor_scalar_mul(
                                ob[:, sqh * 8 + ch, :], acc[:, 0:64], rc[:]
                            )
                            pacer.slot(pace_s2)
                    nc.sync.dma_start(
                        osc.ap()[hl].rearrange("(u t p) d -> p (u t) d", p=P, t=8),
                        ob[:],
                    )
                    # M rows for this head
                    r0 = (hl % 2) * 64
                    nc.sync.dma_start(M_sb[par][ft][r0 : r0 + 64, :], oscM[hl])

                # software-pipelined rep schedule (parity ping-pong):
                #   prologue: P(par 0)
                #   rep r: A(h0..h2, par)+[O_{r-1}] ; A(h3, par)+[P_{r+1}]
                #   epilogue: O of last rep
                pacer = Pacer()
                for _, u in proj_units(0):
                    u()
                for rep in range(reps):
                    par = rep % 2
                    if rep > 0:
                        pacer.add(outproj_units(1 - par, "dve"))
                    attention_head(par, 0, pacer)
                    attention_head(par, 1, pacer)
                    attention_head(par, 2, pacer)
                    if rep < reps - 1:
                        pacer.add(proj_units(1 - par))
                    attention_head(par, 3, pacer)
                    pacer.flush()
                for _, u in outproj_units((reps - 1) % 2, "act"):
                    u()

    nc.compile()
    return nc


_CACHE = {}


def _np_elt(mode=None):
    import ml_dtypes

    return ml_dtypes.bfloat16


def _get_runner(mode, reps=1):
    """Build (once) the compiled kernel + a persistent jitted executor."""
    key = (mode, reps)
    if key in _CACHE:
        return _CACHE[key]

    import jax
    import jax.numpy as jnp  # noqa: F401
    from jax.sharding import Mesh, PartitionSpec
    from jax.experimental.shard_map import shard_map
    import concourse.mybir as mybir
    from concourse import bass2jax

    nc = _build_nc(mode, reps)
    bass2jax.install_neuronx_cc_hook()

    partition_name = (
        nc.partition_id_tensor.name if nc.partition_id_tensor else None
    )
    in_names = []
    out_names = []
    out_avals = []
    for alloc in nc.m.functions[0].allocations:
        if not isinstance(alloc, mybir.MemoryLocationSet):
            continue
        name = alloc.memorylocations[0].name
        if alloc.kind == "ExternalInput":
            if name != partition_name:
                in_names.append(name)
        elif alloc.kind == "ExternalOutput":
            out_names.append(name)
            shape = tuple(alloc.tensor_shape)
            dtype = mybir.dt.np(alloc.dtype)
            out_avals.append(jax.core.ShapedArray(shape, dtype))
    n_params = len(in_names)
    n_outs = len(out_avals)
    all_in_names = list(in_names) + list(out_names)
    if partition_name is not None:
        all_in_names.append(partition_name)
    all_in_names = tuple(all_in_names)

    def _body(*args):
        operands = list(args)
        if partition_name is not None:
            operands.append(bass2jax.partition_id_tensor())
        outs = bass2jax._bass_exec_p.bind(
            *operands,
            out_avals=tuple(out_avals),
            in_names=all_in_names,
            out_names=tuple(out_names),
            lowering_input_output_aliases=(),
            sim_require_finite=True,
            sim_require_nnan=True,
            nc=nc,
        )
        return tuple(outs)

    devices = jax.devices()[:N_CORES]
    mesh = Mesh(np.asarray(devices), ("core",))
    in_specs = (PartitionSpec("core"),) * (n_params + n_outs)
    out_specs = (PartitionSpec("core"),) * n_outs
    donate = tuple(range(n_params, n_params + n_outs))
    sharded = jax.jit(
        shard_map(
            _body, mesh=mesh, in_specs=in_specs, out_specs=out_specs,
            check_rep=False,
        ),
        donate_argnums=donate,
        keep_unused=True,
    )

    zero_out_shapes = [
        ((N_CORES * a.shape[0],) + tuple(a.shape[1:]), a.dtype)
        for a in out_avals
    ]

    def execute(in_maps):
        concat_in = [
            np.concatenate([np.asarray(m[name]) for m in in_maps], axis=0)
            for name in in_names
        ]
        concat_zeros = [np.zeros(s, d) for s, d in zero_out_shapes]
        out_arrs = sharded(*concat_in, *concat_zeros)
        out_arrs = [np.asarray(o) for o in out_arrs]
        return [
            {
                name: out_arrs[i].reshape(
                    N_CORES, *out_avals[i].shape
                )[c]
                for i, name in enumerate(out_names)
            }
            for c in range(N_CORES)
        ]

    execute.in_names = in_names
    execute.out_names = out_names
    execute.out_avals = out_avals
    execute.n_params = n_params
    execute.body = _body
    execute.mesh = mesh
    execute.zero_out_shapes = zero_out_shapes
    _CACHE[key] = execute
    return execute


def make_in_maps(x, Wq, bq, Wk, bk, Wv, bv, Wo, bo, mode=None):
    ne = _np_elt()
    x = np.asarray(x, np.float32)
    in_maps = []
    for c in range(N_CORES):
        b = c // 4
        sl = slice((c % 4) * NF, (c % 4) * NF + NF)
        xT = np.ascontiguousarray(x[b].T).astype(ne)
        in_maps.append(
            {
                "xT": xT,
                "wqT": np.ascontiguousarray(
                    np.asarray(Wq)[sl, :].T * SCALE
                ).astype(ne),
                "wkT": np.ascontiguousarray(np.asarray(Wk)[sl, :].T).astype(ne),
                "wvT": np.ascontiguousarray(np.asarray(Wv)[sl, :].T).astype(ne),
                "woT": np.ascontiguousarray(np.asarray(Wo)[:, sl].T).astype(ne),
                "bqc": np.ascontiguousarray(
                    (np.asarray(bq, np.float32)[sl] * SCALE).reshape(2, P).T
                ),
                "bkc": np.ascontiguousarray(
                    np.asarray(bk, np.float32)[sl].reshape(2, P).T
                ),
                "bvr": np.asarray(bv, np.float32)[sl].reshape(1, NF).astype(ne),
            }
        )
    return in_maps


def kernel(x, Wq, bq, Wk, bk, Wv, bv, Wo, bo):
    mode = DTYPE_MODE
    execute = _get_runner(mode)
    in_maps = make_in_maps(x, Wq, bq, Wk, bk, Wv, bv, Wo, bo, mode)
    results = execute(in_maps)
    ysum = np.zeros((B, D, S), np.float64)
    for c in range(N_CORES):
        ysum[c // 4] += np.asarray(results[c]["ypT"], np.float32)
    y = ysum.transpose(0, 2, 1) + np.asarray(bo, np.float32)[None, None, :]
    return np.ascontiguousarray(y.astype(np.float32))
